# revision 20
# baseline (speedup 1.0000x reference)
"""Edge-parallel GNN u_mul_v kernel for Trainium2 (8 NeuronCores).

z[e, :] = h[src[e], :] * h[dst[e], :]

Sharding: edges are globally sorted by src and cores take contiguous 100K
spans, so each core's src values live in a narrow ~6.3K-node window. That
window of h is shipped per-core as a rebased table (hsrc), so src indices
fit the gather's signed-int16 format and the src gather only touches a
~0.8MB hot set of HBM.

Tables are dense fp16: two nodes packed per 256B row (h16 reshaped to
[N_PAD/2, 128]). A gather fetches one node = 128 bytes: elem_size=64 fp16
with elem_step=128 (256B row stride, the SWDGE stride granularity), index =
node//2, and the node's parity selects the even/odd 128B column view of the
table. The bass dma_gather wrapper refuses sub-256B elements (a transpose-
path XBAR restriction), but the non-transpose TX/RX descriptors
(q7_kernels/extended_inst/dma_gather.cpp) carry arbitrary lengths, so
_gather128 builds InstDMAGatherAnt directly. 128B descriptors cost half of
256B ones (both are under the 512B full-rate threshold) and halve the HBM
bytes fetched per edge.

Per core, edges are bucketed by (src parity, dst parity) and sorted by dst
inside each bucket: the dst gather sweeps the packed table near-
sequentially instead of randomly. Device programs are SPMD so bucket
capacities are padded to the per-bucket max across cores.

Per 8192-edge tile: two SWDGE dma_gathers (src/dst on separate queues), one
DVE fp16 multiply, one contiguous HWDGE store of z [E_DEV, 64] fp16 (the
host converts to fp32 and applies the inverse edge permutation). Gather
index tensors are shipped once (not 8x-replicated): queue q's Q7 pair only
reads idx words from partitions [32q, 32q+32).
"""

import numpy as np

N_NODES = 50000
N_EDGES = 800000
D = 64
N_CORES = 8
E_PER_CORE = N_EDGES // N_CORES  # 100000
N_PAD = -(-N_NODES // 128) * 128  # 50048 table rows
W_SRC = 8192  # per-core src window (nodes; window is ~6.3K wide)
NI = 8192  # edges per tile (per dma_gather call; 513 descs < 1024 ring)

_cached = {}  # tiles structure -> compiled nc


def _gather128(nc, out_ap, in_ap, idxs_ap, ni, queue_num):
    """Non-transpose SWDGE gather of 128-byte fp16 rows from a 256B-stride
    DRAM table (an even/odd column view of the dense packed table)."""
    from concourse import mybir

    eng = nc.gpsimd
    eng._assert_queue_num(queue_num)
    elem_size = D  # fp16 elements = 128B
    elem_step = 2 * D  # 256B row stride
    assert in_ap.dtype == out_ap.dtype == mybir.dt.float16
    assert in_ap.ap[0][0] == elem_step
    assert in_ap.ap[-1][1] == out_ap.ap[-1][1] == elem_size
    assert out_ap.ap[0][1] * out_ap.ap[1][1] == ni
    _in_ap = eng.lower_ap_dma(in_ap, for_custom_bir_dma=True)
    return eng.add_instruction(
        mybir.InstDMAGatherAnt(
            name=nc.get_next_instruction_name(),
            ins=[
                *_in_ap,
                eng.lower_ap(idxs_ap),
                eng.lower_val_access(eng.to_reg(ni)),
            ],
            outs=[eng.lower_ap(out_ap)],
            transpose=False,
            num_idxs=ni,
            elem_size=elem_size,
            stride_bytes_256=1,
            gen_mode=0,
            single_packet=False,
            queue_num=queue_num,
            sbuf_tokens_per_rank=0,
            sbuf_free_dim_per_rank=0,
            sbuf_free_dim_pad_per_rank=0,
            sbuf_byte_offset=0,
        )
    )


def _build(tiles):
    """tiles: list of (src_par, dst_par, ni) per tile (ni % 128 == 0)."""
    import concourse.bass as bass
    import concourse.tile as tile
    from concourse import bacc, mybir

    T = len(tiles)
    E_DEV = sum(t[2] for t in tiles)
    nc = bacc.Bacc(
        "TRN2",
        target_bir_lowering=False,
        debug=False,
        num_devices=N_CORES,
        num_swdge_queues=4,
    )
    h_ap = nc.dram_tensor(
        "h", [N_PAD // 2, 2 * D], mybir.dt.float16, kind="ExternalInput"
    ).ap()
    hs_ap = nc.dram_tensor(
        "hsrc", [W_SRC // 2, 2 * D], mybir.dt.float16, kind="ExternalInput"
    ).ap()
    si_ap = nc.dram_tensor(
        "src_idx", [T, 32, NI // 16], mybir.dt.int16, kind="ExternalInput"
    ).ap()
    di_ap = nc.dram_tensor(
        "dst_idx", [T, 32, NI // 16], mybir.dt.int16, kind="ExternalInput"
    ).ap()
    z_ap = nc.dram_tensor("z", [E_DEV, D], mybir.dt.float16, kind="ExternalOutput").ap()

    dtab = {0: h_ap[:, 0:D], 1: h_ap[:, D : 2 * D]}
    stab = {0: hs_ap[:, 0:D], 1: hs_ap[:, D : 2 * D]}

    with tile.TileContext(nc) as tc:
        with (
            tc.tile_pool(name="ix", bufs=10) as ixp,
            tc.tile_pool(name="ga", bufs=6) as gap,
            tc.tile_pool(name="gb", bufs=6) as gbp,
            tc.tile_pool(name="zt", bufs=6) as ztp,
        ):
            base = 0
            for t, (s_par, d_par, ni) in enumerate(tiles):
                g = ni // 128
                # each gather's Q7 pair (queue q: RX cpu 2q, TX cpu 2q+1)
                # reads idx only from partitions [32q, 32q+32)
                qs = (t % 2) * 2
                six = ixp.tile([128, ni // 16], mybir.dt.int16, tag="six")
                nc.sync.dma_start(
                    six[32 * qs : 32 * qs + 32, :], si_ap[t][:, : ni // 16]
                )
                dix = ixp.tile([128, ni // 16], mybir.dt.int16, tag="dix")
                nc.sync.dma_start(
                    dix[32 * qs + 32 : 32 * qs + 64, :], di_ap[t][:, : ni // 16]
                )
                ga = gap.tile([128, g, D], mybir.dt.float16, tag="ga")
                _gather128(nc, ga[:], stab[s_par], six[:], ni, qs)
                gb = gbp.tile([128, g, D], mybir.dt.float16, tag="gb")
                _gather128(nc, gb[:], dtab[d_par], dix[:], ni, qs + 1)
                zt = ztp.tile([128, g, D], mybir.dt.float16, tag="zt")
                nc.vector.tensor_mul(zt[:], ga[:], gb[:])
                # device z rows [base : base+ni): slot p*g+gg holds gathered
                # position gg*128+p; contiguous per partition (g*128B runs)
                z_view = z_ap[base : base + ni, :].rearrange(
                    "(p gd) d -> p (gd d)", p=128
                )
                nc.sync.dma_start(z_view, zt[:])
                base += ni
    nc.compile()
    return nc


def _wrap16(a):
    """[ni] int16 gather-sequence -> wrapped [32, ni//16] layout:
    position i lives at partition i%16, slot i//16, replicated x2 (one copy
    for each Q7 core of the queue's RX/TX pair)."""
    w = a.reshape(-1, 16).T
    return np.ascontiguousarray(np.tile(w, (2, 1)))


def _prepare(src, dst):
    """Globally sort edges by src; shard contiguous spans; per core bucket by
    (src parity, dst parity) and sort by dst inside each bucket. Build
    per-core packed int16 pair-index tensors (src rebased to the core's
    even-aligned window), the shared tile structure, per-core window bases,
    and the device-order -> edge map."""
    src = np.asarray(src).astype(np.int64)
    dst = np.asarray(dst).astype(np.int64)
    order = np.argsort(src, kind="stable")
    spans = [order[c * E_PER_CORE : (c + 1) * E_PER_CORE] for c in range(N_CORES)]
    n0s = [int(src[sp].min()) & ~1 for sp in spans]
    groups = []  # [core][k] -> original edge ids, dst-sorted
    for c in range(N_CORES):
        e = spans[c]
        k = (src[e] & 1) * 2 + (dst[e] & 1)
        glist = []
        for kk in range(4):
            ee = e[k == kk]
            ee = ee[np.argsort(dst[ee], kind="stable")]
            glist.append(ee)
        groups.append(glist)
        assert src[e].max() - n0s[c] < W_SRC
    caps = [
        -(-max(len(groups[c][k]) for c in range(N_CORES)) // 128) * 128
        for k in range(4)
    ]
    tiles = []
    for k in range(4):
        rem = caps[k]
        while rem > 0:
            ni = min(NI, rem)
            tiles.append((k >> 1, k & 1, ni))
            rem -= ni

    def _split(t, sizes):
        s_par, d_par, ni = t
        out, rem = [], ni
        for s in sizes:
            s = min(s, rem)
            if s <= 0:
                continue
            out.append((s_par, d_par, s))
            rem -= s
        if rem > 0:
            out.append((s_par, d_par, rem))
        return out

    # small first/last tiles shorten the pipeline fill (desc-gen before the
    # first DMA) and the store drain at the end
    tiles = (
        _split(tiles[0], [1024, 1024, 2048])
        + tiles[1:-1]
        + _split(tiles[-1], [tiles[-1][2] - 2048, 1024, 1024])
    )
    T = len(tiles)
    E_DEV = sum(t[2] for t in tiles)

    tile_bases = np.cumsum([0] + [t[2] for t in tiles])
    in_maps = []
    dev_orig = np.empty((N_CORES, E_DEV), np.int64)
    for c in range(N_CORES):
        orig = np.full(E_DEV, -1, np.int64)
        pos = 0
        for k in range(4):
            e = groups[c][k]
            orig[pos : pos + len(e)] = e
            pos += caps[k]
        s_loc = (src[np.maximum(orig, 0)] - n0s[c]) >> 1
        d_loc = dst[np.maximum(orig, 0)] >> 1
        si = np.zeros((T, 32, NI // 16), np.int16)
        di = np.zeros((T, 32, NI // 16), np.int16)
        for t, (s_par, d_par, ni) in enumerate(tiles):
            b = tile_bases[t]
            s16 = np.where(orig[b : b + ni] >= 0, s_loc[b : b + ni], 0).astype(
                np.int16
            )
            d16 = np.where(orig[b : b + ni] >= 0, d_loc[b : b + ni], 0).astype(
                np.int16
            )
            si[t, :, : ni // 16] = _wrap16(s16)
            di[t, :, : ni // 16] = _wrap16(d16)
            # device slot p*(ni//128)+g holds gathered position g*128+p
            tmap = np.arange(ni).reshape(ni // 128, 128).T.reshape(-1)
            dev_orig[c, b : b + ni] = orig[b : b + ni][tmap]
        in_maps.append({"si": si, "di": di})
    return tiles, in_maps, dev_orig, n0s


def _get_nc(tiles):
    key = tuple(tiles)
    if key not in _cached:
        _cached[key] = _build(list(key))
    return _cached[key]


def _make_in_maps(h, src, dst):
    tiles, idx_maps, dev_orig, n0s = _prepare(src, dst)
    h16 = np.asarray(h, dtype=np.float16)
    hpk = np.zeros((N_PAD // 2, 2 * D), np.float16)
    hpk[: N_NODES // 2] = h16.reshape(N_NODES // 2, 2 * D)
    in_maps = []
    for c, m in enumerate(idx_maps):
        hs = np.zeros((W_SRC // 2, 2 * D), np.float16)
        end = min(n0s[c] + W_SRC, N_NODES)
        n = end - n0s[c]
        flat = np.zeros((W_SRC, D), np.float16)
        flat[:n] = h16[n0s[c] : end]
        hs[:] = flat.reshape(W_SRC // 2, 2 * D)
        in_maps.append(
            {"h": hpk, "hsrc": hs, "src_idx": m["si"], "dst_idx": m["di"]}
        )
    return tiles, in_maps, dev_orig


def kernel(h, src, dst):
    from concourse import bass_utils

    tiles, in_maps, dev_orig = _make_in_maps(h, src, dst)
    nc = _get_nc(tiles)
    res = bass_utils.run_bass_kernel_spmd(nc, in_maps, list(range(N_CORES)))
    out = np.empty((N_EDGES, D), np.float32)
    for c in range(N_CORES):
        zc = res.results[c]["z"]
        valid = dev_orig[c] >= 0
        out[dev_orig[c][valid]] = zc[valid].astype(np.float32)
    return out


# revision 22
# speedup vs baseline: 1.0059x; 1.0059x over previous
"""Edge-parallel GNN u_mul_v kernel for Trainium2 (8 NeuronCores).

z[e, :] = h[src[e], :] * h[dst[e], :]

Sharding: edges are globally sorted by src and cores take contiguous 100K
spans, so each core's src values live in a narrow ~6.3K-node window. That
window of h is shipped per-core as a rebased table (hsrc), so src indices
fit the gather's signed-int16 format and the src gather only touches a
~0.8MB hot set of HBM.

Tables are dense fp16: two nodes packed per 256B row (h16 reshaped to
[N_PAD/2, 128]). A gather fetches one node = 128 bytes: elem_size=64 fp16
with elem_step=128 (256B row stride, the SWDGE stride granularity), index =
node//2, and the node's parity selects the even/odd 128B column view of the
table. The bass dma_gather wrapper refuses sub-256B elements (a transpose-
path XBAR restriction), but the non-transpose TX/RX descriptors
(q7_kernels/extended_inst/dma_gather.cpp) carry arbitrary lengths, so
_gather128 builds InstDMAGatherAnt directly. 128B descriptors cost half of
256B ones (both are under the 512B full-rate threshold) and halve the HBM
bytes fetched per edge.

Per core, edges are bucketed by (src parity, dst parity) and sorted by dst
inside each bucket: the dst gather sweeps the packed table near-
sequentially instead of randomly. Device programs are SPMD so bucket
capacities are padded to the per-bucket max across cores.

Per 8192-edge tile: two SWDGE dma_gathers (src/dst on separate queues), one
DVE fp16 multiply, one contiguous HWDGE store of z [E_DEV, 64] fp16 (the
host converts to fp32 and applies the inverse edge permutation). Gather
index tensors are shipped once (not 8x-replicated): queue q's Q7 pair only
reads idx words from partitions [32q, 32q+32).
"""

import numpy as np

N_NODES = 50000
N_EDGES = 800000
D = 64
N_CORES = 8
E_PER_CORE = N_EDGES // N_CORES  # 100000
N_PAD = -(-N_NODES // 128) * 128  # 50048 table rows
W_SRC = 8192  # per-core src window (nodes; window is ~6.3K wide)
NI = 8192  # edges per tile (per dma_gather call; 513 descs < 1024 ring)

_cached = {}  # tiles structure -> compiled nc


def _gather128(nc, out_ap, in_ap, idxs_ap, ni, queue_num):
    """Non-transpose SWDGE gather of 128-byte fp16 rows from a 256B-stride
    DRAM table (an even/odd column view of the dense packed table)."""
    from concourse import mybir

    eng = nc.gpsimd
    eng._assert_queue_num(queue_num)
    elem_size = D  # fp16 elements = 128B
    elem_step = 2 * D  # 256B row stride
    assert in_ap.dtype == out_ap.dtype == mybir.dt.float16
    assert in_ap.ap[0][0] == elem_step
    assert in_ap.ap[-1][1] == out_ap.ap[-1][1] == elem_size
    assert out_ap.ap[0][1] * out_ap.ap[1][1] == ni
    _in_ap = eng.lower_ap_dma(in_ap, for_custom_bir_dma=True)
    return eng.add_instruction(
        mybir.InstDMAGatherAnt(
            name=nc.get_next_instruction_name(),
            ins=[
                *_in_ap,
                eng.lower_ap(idxs_ap),
                eng.lower_val_access(eng.to_reg(ni)),
            ],
            outs=[eng.lower_ap(out_ap)],
            transpose=False,
            num_idxs=ni,
            elem_size=elem_size,
            stride_bytes_256=1,
            gen_mode=0,
            single_packet=False,
            queue_num=queue_num,
            sbuf_tokens_per_rank=0,
            sbuf_free_dim_per_rank=0,
            sbuf_free_dim_pad_per_rank=0,
            sbuf_byte_offset=0,
        )
    )


def _build(tiles):
    """tiles: list of (src_par, dst_par, ni) per tile (ni % 128 == 0)."""
    import concourse.bass as bass
    import concourse.tile as tile
    from concourse import bacc, mybir

    T = len(tiles)
    E_DEV = sum(t[2] for t in tiles)
    nc = bacc.Bacc(
        "TRN2",
        target_bir_lowering=False,
        debug=False,
        num_devices=N_CORES,
        num_swdge_queues=4,
    )
    h_ap = nc.dram_tensor(
        "h", [N_PAD // 2, 2 * D], mybir.dt.float16, kind="ExternalInput"
    ).ap()
    hs_ap = nc.dram_tensor(
        "hsrc", [W_SRC // 2, 2 * D], mybir.dt.float16, kind="ExternalInput"
    ).ap()
    si_ap = nc.dram_tensor(
        "src_idx", [T, 32, NI // 16], mybir.dt.int16, kind="ExternalInput"
    ).ap()
    di_ap = nc.dram_tensor(
        "dst_idx", [T, 32, NI // 16], mybir.dt.int16, kind="ExternalInput"
    ).ap()
    z_ap = nc.dram_tensor("z", [E_DEV, D], mybir.dt.float16, kind="ExternalOutput").ap()

    dtab = {0: h_ap[:, 0:D], 1: h_ap[:, D : 2 * D]}
    stab = {0: hs_ap[:, 0:D], 1: hs_ap[:, D : 2 * D]}

    with tile.TileContext(nc) as tc:
        with (
            tc.tile_pool(name="ix", bufs=10) as ixp,
            tc.tile_pool(name="ga", bufs=6) as gap,
            tc.tile_pool(name="gb", bufs=6) as gbp,
            tc.tile_pool(name="zt", bufs=6) as ztp,
        ):
            base = 0
            for t, (s_par, d_par, ni) in enumerate(tiles):
                g = ni // 128
                # each gather's Q7 pair (queue q: RX cpu 2q, TX cpu 2q+1)
                # reads idx only from partitions [32q, 32q+32)
                qs = (t % 2) * 2
                six = ixp.tile([128, ni // 16], mybir.dt.int16, tag="six")
                nc.sync.dma_start(
                    six[32 * qs : 32 * qs + 32, :], si_ap[t][:, : ni // 16]
                )
                dix = ixp.tile([128, ni // 16], mybir.dt.int16, tag="dix")
                nc.sync.dma_start(
                    dix[32 * qs + 32 : 32 * qs + 64, :], di_ap[t][:, : ni // 16]
                )
                ga = gap.tile([128, g, D], mybir.dt.float16, tag="ga")
                _gather128(nc, ga[:], stab[s_par], six[:], ni, qs)
                gb = gbp.tile([128, g, D], mybir.dt.float16, tag="gb")
                _gather128(nc, gb[:], dtab[d_par], dix[:], ni, qs + 1)
                zt = ztp.tile([128, g, D], mybir.dt.float16, tag="zt")
                nc.vector.tensor_mul(zt[:], ga[:], gb[:])
                # device z rows [base : base+ni): slot p*g+gg holds gathered
                # position gg*128+p; contiguous per partition (g*128B runs)
                z_view = z_ap[base : base + ni, :].rearrange(
                    "(p gd) d -> p (gd d)", p=128
                )
                nc.sync.dma_start(z_view, zt[:])
                base += ni
    nc.compile()
    return nc


def _wrap16(a):
    """[ni] int16 gather-sequence -> wrapped [32, ni//16] layout:
    position i lives at partition i%16, slot i//16, replicated x2 (one copy
    for each Q7 core of the queue's RX/TX pair)."""
    w = a.reshape(-1, 16).T
    return np.ascontiguousarray(np.tile(w, (2, 1)))


def _prepare(src, dst):
    """Globally sort edges by src; shard contiguous spans; per core bucket by
    (src parity, dst parity) and sort by dst inside each bucket. Build
    per-core packed int16 pair-index tensors (src rebased to the core's
    even-aligned window), the shared tile structure, per-core window bases,
    and the device-order -> edge map."""
    src = np.asarray(src).astype(np.int64)
    dst = np.asarray(dst).astype(np.int64)
    order = np.argsort(src, kind="stable")
    spans = [order[c * E_PER_CORE : (c + 1) * E_PER_CORE] for c in range(N_CORES)]
    n0s = [int(src[sp].min()) & ~1 for sp in spans]
    groups = []  # [core][k] -> original edge ids, dst-sorted
    for c in range(N_CORES):
        e = spans[c]
        k = (src[e] & 1) * 2 + (dst[e] & 1)
        glist = []
        for kk in range(4):
            ee = e[k == kk]
            ee = ee[np.argsort(dst[ee], kind="stable")]
            glist.append(ee)
        groups.append(glist)
        assert src[e].max() - n0s[c] < W_SRC
    caps = [
        -(-max(len(groups[c][k]) for c in range(N_CORES)) // 128) * 128
        for k in range(4)
    ]
    tiles = []
    for k in range(4):
        rem = caps[k]
        while rem > 0:
            ni = min(NI, rem)
            tiles.append((k >> 1, k & 1, ni))
            rem -= ni
    # a short first tile reaches the first gather DMA ~4us sooner (desc-gen
    # is 994ns + 0.34ns/idx, serial before the first transfer); a short last
    # tile shrinks the unoverlapped final store
    if tiles[0][2] == NI:
        s, d, _ = tiles[0]
        tiles[0:1] = [(s, d, 2048), (s, d, NI - 2048)]
    if tiles[-1][2] > 4096:
        s, d, ni = tiles[-1]
        tiles[-1:] = [(s, d, ni - 2048), (s, d, 2048)]

    T = len(tiles)
    E_DEV = sum(t[2] for t in tiles)

    tile_bases = np.cumsum([0] + [t[2] for t in tiles])
    in_maps = []
    dev_orig = np.empty((N_CORES, E_DEV), np.int64)
    for c in range(N_CORES):
        orig = np.full(E_DEV, -1, np.int64)
        pos = 0
        for k in range(4):
            e = groups[c][k]
            orig[pos : pos + len(e)] = e
            pos += caps[k]
        s_loc = (src[np.maximum(orig, 0)] - n0s[c]) >> 1
        d_loc = dst[np.maximum(orig, 0)] >> 1
        si = np.zeros((T, 32, NI // 16), np.int16)
        di = np.zeros((T, 32, NI // 16), np.int16)
        for t, (s_par, d_par, ni) in enumerate(tiles):
            b = tile_bases[t]
            s16 = np.where(orig[b : b + ni] >= 0, s_loc[b : b + ni], 0).astype(
                np.int16
            )
            d16 = np.where(orig[b : b + ni] >= 0, d_loc[b : b + ni], 0).astype(
                np.int16
            )
            si[t, :, : ni // 16] = _wrap16(s16)
            di[t, :, : ni // 16] = _wrap16(d16)
            # device slot p*(ni//128)+g holds gathered position g*128+p
            tmap = np.arange(ni).reshape(ni // 128, 128).T.reshape(-1)
            dev_orig[c, b : b + ni] = orig[b : b + ni][tmap]
        in_maps.append({"si": si, "di": di})
    return tiles, in_maps, dev_orig, n0s


def _get_nc(tiles):
    key = tuple(tiles)
    if key not in _cached:
        _cached[key] = _build(list(key))
    return _cached[key]


def _make_in_maps(h, src, dst):
    tiles, idx_maps, dev_orig, n0s = _prepare(src, dst)
    h16 = np.asarray(h, dtype=np.float16)
    hpk = np.zeros((N_PAD // 2, 2 * D), np.float16)
    hpk[: N_NODES // 2] = h16.reshape(N_NODES // 2, 2 * D)
    in_maps = []
    for c, m in enumerate(idx_maps):
        hs = np.zeros((W_SRC // 2, 2 * D), np.float16)
        end = min(n0s[c] + W_SRC, N_NODES)
        n = end - n0s[c]
        flat = np.zeros((W_SRC, D), np.float16)
        flat[:n] = h16[n0s[c] : end]
        hs[:] = flat.reshape(W_SRC // 2, 2 * D)
        in_maps.append(
            {"h": hpk, "hsrc": hs, "src_idx": m["si"], "dst_idx": m["di"]}
        )
    return tiles, in_maps, dev_orig


def kernel(h, src, dst):
    from concourse import bass_utils

    tiles, in_maps, dev_orig = _make_in_maps(h, src, dst)
    nc = _get_nc(tiles)
    res = bass_utils.run_bass_kernel_spmd(nc, in_maps, list(range(N_CORES)))
    out = np.empty((N_EDGES, D), np.float32)
    for c in range(N_CORES):
        zc = res.results[c]["z"]
        valid = dev_orig[c] >= 0
        out[dev_orig[c][valid]] = zc[valid].astype(np.float32)
    return out


# revision 23
# speedup vs baseline: 1.0062x; 1.0004x over previous
"""Edge-parallel GNN u_mul_v kernel for Trainium2 (8 NeuronCores).

z[e, :] = h[src[e], :] * h[dst[e], :]

Sharding: edges are globally sorted by src and cores take contiguous 100K
spans, so each core's src values live in a narrow ~6.3K-node window. That
window of h is shipped per-core as a rebased table (hsrc), so src indices
fit the gather's signed-int16 format and the src gather only touches a
~0.8MB hot set of HBM.

Tables are dense fp16: two nodes packed per 256B row (h16 reshaped to
[N_PAD/2, 128]). A gather fetches one node = 128 bytes: elem_size=64 fp16
with elem_step=128 (256B row stride, the SWDGE stride granularity), index =
node//2, and the node's parity selects the even/odd 128B column view of the
table. The bass dma_gather wrapper refuses sub-256B elements (a transpose-
path XBAR restriction), but the non-transpose TX/RX descriptors
(q7_kernels/extended_inst/dma_gather.cpp) carry arbitrary lengths, so
_gather128 builds InstDMAGatherAnt directly. 128B descriptors cost half of
256B ones (both are under the 512B full-rate threshold) and halve the HBM
bytes fetched per edge.

Per core, edges are bucketed by (src parity, dst parity) and sorted by dst
inside each bucket: the dst gather sweeps the packed table near-
sequentially instead of randomly. Device programs are SPMD so bucket
capacities are padded to the per-bucket max across cores.

Per 8192-edge tile: two SWDGE dma_gathers (src/dst on separate queues), one
DVE fp16 multiply, one contiguous HWDGE store of z [E_DEV, 64] fp16 (the
host converts to fp32 and applies the inverse edge permutation). Gather
index tensors are shipped once (not 8x-replicated): queue q's Q7 pair only
reads idx words from partitions [32q, 32q+32).
"""

import numpy as np

N_NODES = 50000
N_EDGES = 800000
D = 64
N_CORES = 8
E_PER_CORE = N_EDGES // N_CORES  # 100000
N_PAD = -(-N_NODES // 128) * 128  # 50048 table rows
W_SRC = 8192  # per-core src window (nodes; window is ~6.3K wide)
NI = 8192  # edges per tile (per dma_gather call; 513 descs < 1024 ring)

_cached = {}  # tiles structure -> compiled nc


def _gather128(nc, out_ap, in_ap, idxs_ap, ni, queue_num):
    """Non-transpose SWDGE gather of 128-byte fp16 rows from a 256B-stride
    DRAM table (an even/odd column view of the dense packed table)."""
    from concourse import mybir

    eng = nc.gpsimd
    eng._assert_queue_num(queue_num)
    elem_size = D  # fp16 elements = 128B
    elem_step = 2 * D  # 256B row stride
    assert in_ap.dtype == out_ap.dtype == mybir.dt.float16
    assert in_ap.ap[0][0] == elem_step
    assert in_ap.ap[-1][1] == out_ap.ap[-1][1] == elem_size
    assert out_ap.ap[0][1] * out_ap.ap[1][1] == ni
    _in_ap = eng.lower_ap_dma(in_ap, for_custom_bir_dma=True)
    return eng.add_instruction(
        mybir.InstDMAGatherAnt(
            name=nc.get_next_instruction_name(),
            ins=[
                *_in_ap,
                eng.lower_ap(idxs_ap),
                eng.lower_val_access(eng.to_reg(ni)),
            ],
            outs=[eng.lower_ap(out_ap)],
            transpose=False,
            num_idxs=ni,
            elem_size=elem_size,
            stride_bytes_256=1,
            gen_mode=0,
            single_packet=False,
            queue_num=queue_num,
            sbuf_tokens_per_rank=0,
            sbuf_free_dim_per_rank=0,
            sbuf_free_dim_pad_per_rank=0,
            sbuf_byte_offset=0,
        )
    )


def _build(tiles):
    """tiles: list of (src_par, dst_par, ni) per tile (ni % 128 == 0)."""
    import concourse.bass as bass
    import concourse.tile as tile
    from concourse import bacc, mybir

    T = len(tiles)
    E_DEV = sum(t[2] for t in tiles)
    nc = bacc.Bacc(
        "TRN2",
        target_bir_lowering=False,
        debug=False,
        num_devices=N_CORES,
        num_swdge_queues=4,
    )
    h_ap = nc.dram_tensor(
        "h", [N_PAD // 2, 2 * D], mybir.dt.float16, kind="ExternalInput"
    ).ap()
    hs_ap = nc.dram_tensor(
        "hsrc", [W_SRC // 2, 2 * D], mybir.dt.float16, kind="ExternalInput"
    ).ap()
    si_ap = nc.dram_tensor(
        "src_idx", [T, 32, NI // 16], mybir.dt.int16, kind="ExternalInput"
    ).ap()
    di_ap = nc.dram_tensor(
        "dst_idx", [T, 32, NI // 16], mybir.dt.int16, kind="ExternalInput"
    ).ap()
    z_ap = nc.dram_tensor("z", [E_DEV, D], mybir.dt.float16, kind="ExternalOutput").ap()

    dtab = {0: h_ap[:, 0:D], 1: h_ap[:, D : 2 * D]}
    stab = {0: hs_ap[:, 0:D], 1: hs_ap[:, D : 2 * D]}

    with tile.TileContext(nc) as tc:
        with (
            tc.tile_pool(name="ix", bufs=10) as ixp,
            tc.tile_pool(name="ga", bufs=6) as gap,
            tc.tile_pool(name="gb", bufs=6) as gbp,
            tc.tile_pool(name="zt", bufs=6) as ztp,
        ):
            base = 0
            for t, (s_par, d_par, ni) in enumerate(tiles):
                g = ni // 128
                # each gather's Q7 pair (queue q: RX cpu 2q, TX cpu 2q+1)
                # reads idx only from partitions [32q, 32q+32)
                qs = (t % 2) * 2
                six = ixp.tile([128, ni // 16], mybir.dt.int16, tag="six")
                nc.sync.dma_start(
                    six[32 * qs : 32 * qs + 32, :], si_ap[t][:, : ni // 16]
                )
                dix = ixp.tile([128, ni // 16], mybir.dt.int16, tag="dix")
                nc.sync.dma_start(
                    dix[32 * qs + 32 : 32 * qs + 64, :], di_ap[t][:, : ni // 16]
                )
                ga = gap.tile([128, g, D], mybir.dt.float16, tag="ga")
                _gather128(nc, ga[:], stab[s_par], six[:], ni, qs)
                gb = gbp.tile([128, g, D], mybir.dt.float16, tag="gb")
                _gather128(nc, gb[:], dtab[d_par], dix[:], ni, qs + 1)
                zt = ztp.tile([128, g, D], mybir.dt.float16, tag="zt")
                nc.vector.tensor_mul(zt[:], ga[:], gb[:])
                # device z rows [base : base+ni): slot p*g+gg holds gathered
                # position gg*128+p; contiguous per partition (g*128B runs)
                z_view = z_ap[base : base + ni, :].rearrange(
                    "(p gd) d -> p (gd d)", p=128
                )
                nc.sync.dma_start(z_view, zt[:])
                base += ni
    nc.compile()
    return nc


def _wrap16(a):
    """[ni] int16 gather-sequence -> wrapped [32, ni//16] layout:
    position i lives at partition i%16, slot i//16, replicated x2 (one copy
    for each Q7 core of the queue's RX/TX pair)."""
    w = a.reshape(-1, 16).T
    return np.ascontiguousarray(np.tile(w, (2, 1)))


def _prepare(src, dst):
    """Globally sort edges by src; shard contiguous spans; per core bucket by
    (src parity, dst parity) and sort by dst inside each bucket. Build
    per-core packed int16 pair-index tensors (src rebased to the core's
    even-aligned window), the shared tile structure, per-core window bases,
    and the device-order -> edge map."""
    src = np.asarray(src).astype(np.int64)
    dst = np.asarray(dst).astype(np.int64)
    order = np.argsort(src, kind="stable")
    spans = [order[c * E_PER_CORE : (c + 1) * E_PER_CORE] for c in range(N_CORES)]
    n0s = [int(src[sp].min()) & ~1 for sp in spans]
    groups = []  # [core][k] -> original edge ids, dst-sorted
    for c in range(N_CORES):
        e = spans[c]
        k = (src[e] & 1) * 2 + (dst[e] & 1)
        glist = []
        for kk in range(4):
            ee = e[k == kk]
            ee = ee[np.argsort(dst[ee], kind="stable")]
            glist.append(ee)
        groups.append(glist)
        assert src[e].max() - n0s[c] < W_SRC
    caps = [
        -(-max(len(groups[c][k]) for c in range(N_CORES)) // 128) * 128
        for k in range(4)
    ]
    tiles = []
    for k in range(4):
        rem = caps[k]
        while rem > 0:
            ni = min(NI, rem)
            tiles.append((k >> 1, k & 1, ni))
            rem -= ni
    T = len(tiles)
    E_DEV = sum(t[2] for t in tiles)

    tile_bases = np.cumsum([0] + [t[2] for t in tiles])
    in_maps = []
    dev_orig = np.empty((N_CORES, E_DEV), np.int64)
    for c in range(N_CORES):
        orig = np.full(E_DEV, -1, np.int64)
        pos = 0
        for k in range(4):
            e = groups[c][k]
            orig[pos : pos + len(e)] = e
            pos += caps[k]
        s_loc = (src[np.maximum(orig, 0)] - n0s[c]) >> 1
        d_loc = dst[np.maximum(orig, 0)] >> 1
        si = np.zeros((T, 32, NI // 16), np.int16)
        di = np.zeros((T, 32, NI // 16), np.int16)
        for t, (s_par, d_par, ni) in enumerate(tiles):
            b = tile_bases[t]
            s16 = np.where(orig[b : b + ni] >= 0, s_loc[b : b + ni], 0).astype(
                np.int16
            )
            d16 = np.where(orig[b : b + ni] >= 0, d_loc[b : b + ni], 0).astype(
                np.int16
            )
            si[t, :, : ni // 16] = _wrap16(s16)
            di[t, :, : ni // 16] = _wrap16(d16)
            # device slot p*(ni//128)+g holds gathered position g*128+p
            tmap = np.arange(ni).reshape(ni // 128, 128).T.reshape(-1)
            dev_orig[c, b : b + ni] = orig[b : b + ni][tmap]
        in_maps.append({"si": si, "di": di})
    return tiles, in_maps, dev_orig, n0s


def _get_nc(tiles):
    key = tuple(tiles)
    if key not in _cached:
        _cached[key] = _build(list(key))
    return _cached[key]


def _make_in_maps(h, src, dst):
    tiles, idx_maps, dev_orig, n0s = _prepare(src, dst)
    h16 = np.asarray(h, dtype=np.float16)
    hpk = np.zeros((N_PAD // 2, 2 * D), np.float16)
    hpk[: N_NODES // 2] = h16.reshape(N_NODES // 2, 2 * D)
    in_maps = []
    for c, m in enumerate(idx_maps):
        hs = np.zeros((W_SRC // 2, 2 * D), np.float16)
        end = min(n0s[c] + W_SRC, N_NODES)
        n = end - n0s[c]
        flat = np.zeros((W_SRC, D), np.float16)
        flat[:n] = h16[n0s[c] : end]
        hs[:] = flat.reshape(W_SRC // 2, 2 * D)
        in_maps.append(
            {"h": hpk, "hsrc": hs, "src_idx": m["si"], "dst_idx": m["di"]}
        )
    return tiles, in_maps, dev_orig


def kernel(h, src, dst):
    from concourse import bass_utils

    tiles, in_maps, dev_orig = _make_in_maps(h, src, dst)
    nc = _get_nc(tiles)
    res = bass_utils.run_bass_kernel_spmd(nc, in_maps, list(range(N_CORES)))
    out = np.empty((N_EDGES, D), np.float32)
    for c in range(N_CORES):
        zc = res.results[c]["z"]
        valid = dev_orig[c] >= 0
        out[dev_orig[c][valid]] = zc[valid].astype(np.float32)
    return out
